# revision 37
# baseline (speedup 1.0000x reference)
"""Multi-Head Latent Attention for Trainium2, sharded over 8 NeuronCores.

Sharding: batch (2) x head-groups (4 of 4 heads each) -> 8 cores.
Host folds the shared down-projection into the per-group query weights
(W_UQ' = W_DQ @ W_UQ[:,g], W_QR' = W_DQ @ W_QR[:,g]), so each core runs:
    phase 1 (one pass over x^T):  c_KV^T, k_rope^T, q_C^T, q_rope^T
    phase 2+3 (per 512-seq block): k_C^T, v_C, then causal flash
        attention on transposed scores [keys, queries]
    phase 4: partial out^T = W_O[g]^T attnout^T
Host sums the 4 head-group partials per batch and transposes back.

All matmuls run in bf16 (1 PE cycle/row, any N) with f32 PSUM
accumulation; every intermediate stays SBUF-resident (no DRAM scratch).
Softmax denominators accumulate in PSUM via a ones-column matmul; the
reciprocal is broadcast across partitions with a K=1 ones-row matmul
instead of a DRAM bounce.
"""
import numpy as np

import concourse.bass as bass
import concourse.mybir as mybir
import concourse.tile as tile
from concourse import bacc
from concourse.bass_utils import run_bass_kernel_spmd

F32 = mybir.dt.float32
BF16 = mybir.dt.bfloat16
Exp = mybir.ActivationFunctionType.Exp
Copy = mybir.ActivationFunctionType.Copy
Mult = mybir.AluOpType.mult

B, S, E = 2, 2048, 2048
H = 16
DH = 128
LOW = 512
R = 64
BASE = 10000.0
HPG = 4               # heads per group (per core)
GCOL = HPG * DH       # 512 columns of this group's heads
P = 128
KE = E // P           # 16 k-tiles over E
KL = LOW // P         # 4 k-tiles over LOW
ST = S // P           # 16 seq tiles of 128
SBN = S // 512        # 4 seq blocks of 512
NEG = -3.0e38
SCALE = 1.0 / float(np.sqrt(DH + R))

_CACHE = {}


def _lhsT_layout(w):
    """[K, M] -> [MT, 128, KT, 128] so slice [mo] is an SBUF tile
    [128p, KT, 128m] with element [p, ko, m] = w[ko*128+p, mo*128+m]."""
    K, M = w.shape
    return np.ascontiguousarray(
        w.reshape(K // P, P, M // P, P).transpose(2, 1, 0, 3))


def _rhs_layout(w):
    """[K, N] -> [128, KT, N]: element [p, ko, n] = w[ko*128+p, n]."""
    K, N = w.shape
    return np.ascontiguousarray(w.reshape(K // P, P, N).transpose(1, 0, 2))


def _rope_perm_cols(w, rope_dim=R):
    """Permute each rope_dim-column block to [evens, odds] order."""
    K, M = w.shape
    nh = M // rope_dim
    w = w.reshape(K, nh, rope_dim)
    perm = np.concatenate([np.arange(0, rope_dim, 2), np.arange(1, rope_dim, 2)])
    return np.ascontiguousarray(w[:, :, perm].reshape(K, M))


def build_nc():
    nc = bacc.Bacc("TRN2", target_bir_lowering=False, debug=False, num_devices=8)

    xT = nc.dram_tensor("xT", [E, S], BF16, kind="ExternalInput")
    wUQ = nc.dram_tensor("wUQ", [HPG, P, KE * P], BF16, kind="ExternalInput")
    wQR = nc.dram_tensor("wQR", [2, P, KE * P], BF16, kind="ExternalInput")
    wDKV = nc.dram_tensor("wDKV", [KL, P, KE * P], BF16, kind="ExternalInput")
    wUK = nc.dram_tensor("wUK", [HPG, P, KL * P], BF16, kind="ExternalInput")
    wUV = nc.dram_tensor("wUV", [P, KL * GCOL], BF16, kind="ExternalInput")
    wKR = nc.dram_tensor("wKR", [P, KE * R], BF16, kind="ExternalInput")
    wO = nc.dram_tensor("wO", [P, HPG * E], BF16, kind="ExternalInput")
    cosq = nc.dram_tensor("cosq", [R, S], F32, kind="ExternalInput")   # [cos;cos]
    sinq = nc.dram_tensor("sinq", [R, S], F32, kind="ExternalInput")   # [-sin;sin]
    maskin = nc.dram_tensor("maskin", [P, 4 * 512], F32, kind="ExternalInput")

    outT = nc.dram_tensor("outT", [E, S], BF16, kind="ExternalOutput")

    with tile.TileContext(nc) as tc:
        with tc.tile_pool(name="persist", bufs=1) as persist:
            kropeT = persist.tile([R, S], BF16, tag="kropeT")
            t_ones = persist.tile([P, 1], BF16, tag="ones")
            nc.vector.memset(t_ones, 1.0)
            t_onesr = persist.tile([1, P], BF16, tag="onesr")
            nc.vector.memset(t_onesr, 1.0)
            qCT = persist.tile([P, HPG, S], BF16, tag="qCT")
            qrE = persist.tile([R, 2, S], BF16, tag="qrE")   # heads 0,2
            qrO = persist.tile([R, 2, S], BF16, tag="qrO")   # heads 1,3
            cKV = persist.tile([P, KL, S], BF16, tag="cKV")

            # phase-2/3/4 fixed weights: pool opened before phase 1 so their
            # DMAs can fire mid-phase-1 (once the x DMA queue has drained)
            stack_wfix2 = tc.tile_pool(name="wfix2", bufs=1)
            wfix2 = stack_wfix2.__enter__()
            t_wuk = [wfix2.tile([P, KL * P], BF16, tag=f"wuk{mo}",
                                name=f"wuk{mo}") for mo in range(HPG)]
            t_wuv = wfix2.tile([P, KL * GCOL], BF16, tag="wuv")
            t_wo = wfix2.tile([P, HPG * E], BF16, tag="wo")

            def fire_wfix2_dmas():
                for mo in range(HPG):
                    nc.sync.dma_start(out=t_wuk[mo], in_=wUK[mo])
                nc.sync.dma_start(out=t_wuv, in_=wUV[:, :])
                nc.sync.dma_start(out=t_wo, in_=wO[:, :])

            with tc.tile_pool(name="tabp", bufs=1) as tabp:
                t_cos = tabp.tile([R, S], F32, tag="cos")
                t_sin = tabp.tile([R, S], F32, tag="sin")

                def rope_from_psum(pool, psum, base, scol, dst):
                    """dst[...] (64 x 512, bf16) = rope(psum[base:base+64]).

                    psum rows [base:base+32]=x1, [base+32:base+64]=x2 (host
                    permuted weight cols). dst = x*[c;c] + swap(x)*[-s;s].
                    TensorCopy may shift partitions; TensorTensor operands
                    must share a base partition, hence copy-then-mul."""
                    sl = slice(scol, scol + 512)
                    swp = pool.tile([R, 512], F32, tag="swp")
                    nc.vector.tensor_copy(out=swp[0:32, :],
                                          in_=psum[base + 32:base + R, :])
                    nc.vector.tensor_copy(out=swp[32:R, :],
                                          in_=psum[base:base + 32, :])
                    nc.vector.tensor_mul(out=swp, in0=swp, in1=t_sin[:, sl])
                    aln = pool.tile([R, 512], F32, tag="aln")
                    if base == 0:
                        nc.vector.tensor_mul(out=aln, in0=psum[0:R, :],
                                             in1=t_cos[:, sl])
                    else:
                        nc.vector.tensor_copy(out=aln,
                                              in_=psum[base:base + R, :])
                        nc.vector.tensor_mul(out=aln, in0=aln,
                                             in1=t_cos[:, sl])
                    nc.vector.tensor_add(out=dst, in0=aln, in1=swp)

                # ---- Phase 1: one pass over x^T -> c_KV^T, k_rope^T,
                #      q_C^T, q_rope^T (all SBUF-resident) ----
                with (
                    tc.tile_pool(name="xt", bufs=40) as xtp,
                    tc.tile_pool(name="wfix1", bufs=1) as wfix1,
                    tc.tile_pool(name="ropetmp", bufs=1) as ropetmp,
                    tc.tile_pool(name="ps_p1", bufs=6, space="PSUM") as ps_p1,
                    tc.tile_pool(name="ps_kr", bufs=1, space="PSUM") as ps_kr,
                ):
                    # DMA issue order tracks first use: wDKV[0], then the
                    # first x block, then the remaining weights.
                    t_wdkv = [wfix1.tile([P, KE * P], BF16, tag=f"wdkv{mo}",
                                         name=f"wdkv{mo}") for mo in range(KL)]
                    t_wkr = wfix1.tile([P, KE * R], BF16, tag="wkr")
                    t_wuq = [wfix1.tile([P, KE * P], BF16, tag=f"wuq{mo}",
                                        name=f"wuq{mo}") for mo in range(HPG)]
                    t_wqr = [wfix1.tile([P, KE * P], BF16, tag=f"wqr{mo}",
                                        name=f"wqr{mo}") for mo in range(2)]
                    nc.sync.dma_start(out=t_wqr[0], in_=wQR[0])

                    for sb in range(SBN):
                        ssl = slice(sb * 512, (sb + 1) * 512)
                        xts = []
                        for k in range(KE):
                            t = xtp.tile([P, 512], BF16, tag="xt")
                            nc.sync.dma_start(
                                out=t, in_=xT[k * P:(k + 1) * P, ssl])
                            xts.append(t)
                        if sb == 0:
                            nc.sync.dma_start(out=t_wqr[1], in_=wQR[1])
                            nc.sync.dma_start(out=t_wkr, in_=wKR[:, :])
                            nc.sync.dma_start(out=t_cos, in_=cosq[:, :])
                            nc.sync.dma_start(out=t_sin, in_=sinq[:, :])
                            for mo in range(KL):
                                nc.sync.dma_start(out=t_wdkv[mo], in_=wDKV[mo])
                            for mo in range(HPG):
                                nc.sync.dma_start(out=t_wuq[mo], in_=wUQ[mo])
                        elif sb == 2:
                            fire_wfix2_dmas()

                        for mo in range(2):    # q_rope^T (2 heads per chain)
                            psum = ps_p1.tile([P, 512], F32, tag="p")
                            for k in range(KE):
                                nc.tensor.matmul(
                                    psum, t_wqr[mo][:, k * P:(k + 1) * P], xts[k],
                                    start=(k == 0), stop=(k == KE - 1))
                            rope_from_psum(ropetmp, psum, 0, sb * 512,
                                           qrE[:, mo, ssl])
                            rope_from_psum(ropetmp, psum, R, sb * 512,
                                           qrO[:, mo, ssl])
                        # k_rope^T
                        psum = ps_kr.tile([R, 512], F32, tag="pkr")
                        for k in range(KE):
                            nc.tensor.matmul(
                                psum, t_wkr[:, k * R:(k + 1) * R], xts[k],
                                start=(k == 0), stop=(k == KE - 1))
                        rope_from_psum(ropetmp, psum, 0, sb * 512,
                                       kropeT[:, ssl])
                        for mo in range(KL):   # c_KV^T
                            psum = ps_p1.tile([P, 512], F32, tag="p")
                            for k in range(KE):
                                nc.tensor.matmul(
                                    psum, t_wdkv[mo][:, k * P:(k + 1) * P], xts[k],
                                    start=(k == 0), stop=(k == KE - 1))
                            nc.scalar.activation(out=cKV[:, mo, ssl],
                                                 in_=psum, func=Copy)
                        for mo in range(HPG):  # q_C^T
                            psum = ps_p1.tile([P, 512], F32, tag="p")
                            for k in range(KE):
                                nc.tensor.matmul(
                                    psum, t_wuq[mo][:, k * P:(k + 1) * P], xts[k],
                                    start=(k == 0), stop=(k == KE - 1))
                            if mo % 2 == 0:
                                nc.scalar.activation(out=qCT[:, mo, ssl],
                                                     in_=psum, func=Copy)
                            else:
                                nc.vector.tensor_copy(out=qCT[:, mo, ssl],
                                                      in_=psum)

            # ---- Phases 2+3 per seq block: k_C^T, v_C, then attention ----
            stack_p2 = tc.tile_pool(name="persist2", bufs=1)
            persist2 = stack_p2.__enter__()
            kCT = persist2.tile([P, HPG, S], BF16, tag="kCT")
            vC = persist2.tile([P, ST, GCOL], BF16, tag="vC")
            aoT = persist2.tile([P, HPG, S], BF16, tag="aoT")
            with (
                tc.tile_pool(name="wfix3", bufs=1) as wfix3,
                tc.tile_pool(name="att", bufs=4) as att,
                tc.tile_pool(name="attsm", bufs=3) as attsm,
                tc.tile_pool(name="ps_kv", bufs=1, space="PSUM") as ps_kv,
                tc.tile_pool(name="ps_s", bufs=3, space="PSUM") as ps_s,
                tc.tile_pool(name="ps_o", bufs=2, space="PSUM") as ps_o,
                tc.tile_pool(name="ps_d", bufs=2, space="PSUM") as ps_d,
            ):
                t_mask = wfix3.tile([P, 4 * 512], F32, tag="mask")
                nc.sync.dma_start(out=t_mask, in_=maskin[:, :])

                # normalization of head block (h, ssl) is deferred until the
                # next head's score loop has been issued, hiding the
                # recip->copy->broadcast-matmul latency under PE work
                pend = []

                def normalize_pending():
                    h, pssl, psum_o, psum_d = pend.pop()
                    rec32 = attsm.tile([1, 512], F32, tag="rec32")
                    nc.vector.reciprocal(out=rec32, in_=psum_d)
                    rec16 = attsm.tile([1, 512], BF16, tag="rec16")
                    nc.scalar.activation(out=rec16, in_=rec32, func=Copy)
                    psum_bc = ps_s.tile([P, 512], F32, tag="p")
                    nc.tensor.matmul(psum_bc, t_onesr, rec16,
                                     start=True, stop=True)
                    bc16 = attsm.tile([P, 512], BF16, tag="bc16")
                    nc.scalar.activation(out=bc16, in_=psum_bc, func=Copy)
                    nc.vector.tensor_tensor(aoT[:, h, pssl], psum_o,
                                            bc16, Mult)

                for sb in range(SBN):
                    ssl = slice(sb * 512, (sb + 1) * 512)
                    for mo in range(HPG):      # k_C^T for this block
                        psum = ps_kv.tile([P, 512], F32, tag="p")
                        for k in range(KL):
                            nc.tensor.matmul(psum, t_wuk[mo][:, k * P:(k + 1) * P],
                                             cKV[:, k, ssl],
                                             start=(k == 0), stop=(k == KL - 1))
                        if mo % 2 == 0:
                            nc.scalar.activation(out=kCT[:, mo, ssl],
                                                 in_=psum, func=Copy)
                        else:
                            nc.vector.tensor_copy(out=kCT[:, mo, ssl],
                                                  in_=psum)
                    for loc in range(4):       # v_C for this block
                        st = sb * 4 + loc
                        psum = ps_kv.tile([P, GCOL], F32, tag="p")
                        for k in range(KL):
                            nc.tensor.matmul(
                                psum, cKV[:, k, st * P:(st + 1) * P],
                                t_wuv[:, k * GCOL:(k + 1) * GCOL],
                                start=(k == 0), stop=(k == KL - 1))
                        if loc % 2 == 0:
                            nc.scalar.activation(out=vC[:, st, :],
                                                 in_=psum, func=Copy)
                        else:
                            nc.vector.tensor_copy(out=vC[:, st, :], in_=psum)

                    # attention for query block sb over key tiles 0..4sb+3
                    T = 4 * (sb + 1)
                    for h in range(HPG):
                        psum_o = ps_o.tile([P, 512], F32, tag="p")
                        psum_d = ps_d.tile([1, 512], F32, tag="p")
                        for tt in range(T):
                            tsl = slice(tt * P, (tt + 1) * P)
                            r = tt - 4 * sb
                            # diagonal key tiles: queries < 128r of this
                            # block are fully masked -> trim the free dim
                            qo = P * r if r > 0 else 0
                            qs = slice(qo, 512)
                            gqs = slice(sb * 512 + qo, (sb + 1) * 512)
                            hqs = (qrE if h % 2 == 0 else qrO)[:, h // 2, gqs]
                            sg = qo > 0
                            psum_s = ps_s.tile([P, 512], F32, tag="p")
                            nc.tensor.matmul(psum_s[:, qs], kCT[:, h, tsl],
                                             qCT[:, h, gqs],
                                             start=True, stop=False)
                            nc.tensor.matmul(psum_s[:, qs], kropeT[:, tsl],
                                             hqs, start=False, stop=True)
                            expT = att.tile([P, 512], BF16, tag="expT")
                            if r >= 0:
                                masked = attsm.tile([P, 512], BF16,
                                                    tag="masked")
                                nc.vector.tensor_add(
                                    out=masked[:, qs], in0=psum_s[:, qs],
                                    in1=t_mask[:, r * 512 + qo:(r + 1) * 512])
                                nc.scalar.activation(out=expT[:, qs],
                                                     in_=masked[:, qs],
                                                     func=Exp)
                            else:
                                nc.scalar.activation(out=expT, in_=psum_s,
                                                     func=Exp)
                            nc.tensor.matmul(psum_d[:, qs], t_ones,
                                             expT[:, qs],
                                             start=(tt == 0), stop=(tt == T - 1),
                                             skip_group_check=sg)
                            nc.tensor.matmul(psum_o[:, qs],
                                             vC[:, tt, h * DH:(h + 1) * DH],
                                             expT[:, qs],
                                             start=(tt == 0), stop=(tt == T - 1),
                                             skip_group_check=sg)
                        if pend:
                            normalize_pending()
                        pend.append((h, ssl, psum_o, psum_d))
                if pend:
                    normalize_pending()

            # ---- Phase 4: out^T partial = W_O[g]^T @ aoT ----
            with (
                tc.tile_pool(name="oout", bufs=6) as oout,
                tc.tile_pool(name="ps_w", bufs=4, space="PSUM") as ps_w,
            ):
                for sb in range(SBN):
                    ssl = slice(sb * 512, (sb + 1) * 512)
                    for mo in range(KE):
                        psum_w = ps_w.tile([P, 512], F32, tag="p")
                        for k in range(HPG):
                            nc.tensor.matmul(psum_w,
                                             t_wo[:, k * E + mo * P:k * E + (mo + 1) * P],
                                             aoT[:, k, ssl],
                                             start=(k == 0), stop=(k == HPG - 1))
                        ot = oout.tile([P, 512], BF16, tag="oout")
                        if mo % 2 == 0:
                            nc.scalar.activation(out=ot, in_=psum_w, func=Copy)
                        else:
                            nc.vector.tensor_copy(out=ot, in_=psum_w)
                        nc.sync.dma_start(
                            out=outT[mo * P:(mo + 1) * P, ssl], in_=ot)
            stack_p2.__exit__(None, None, None)
            stack_wfix2.__exit__(None, None, None)

    nc.compile()
    return nc


def _host_inputs(inputs):
    """Per-core input maps (host-side sharding + weight folding/pre-tiling)."""
    f32 = np.float32
    bf16 = mybir.dt.np(BF16)
    x = inputs["x"]
    W_DQ = inputs["W_DQ"].astype(f32)
    W_UQ, W_QR = inputs["W_UQ"].astype(f32), inputs["W_QR"].astype(f32)
    W_DKV, W_UK = inputs["W_DKV"].astype(f32), inputs["W_UK"].astype(f32)
    W_KR, W_UV = inputs["W_KR"].astype(f32), inputs["W_UV"].astype(f32)
    W_O = inputs["W_O"].astype(f32)

    # fold the shared down-projection into the query weights
    WUQf = (W_DQ @ W_UQ) * SCALE          # [E, E]
    WQRf = (W_DQ @ W_QR) * SCALE          # [E, R*H]

    # shared across cores
    wDKV_t = _lhsT_layout(W_DKV).astype(bf16).reshape(KL, P, KE * P)
    wKR_t = _rhs_layout(_rope_perm_cols(W_KR)).astype(bf16).reshape(P, KE * R)
    half = R // 2
    freqs = BASE ** (-np.arange(half, dtype=np.float64) / half)
    theta = np.arange(S, dtype=np.float64)[None, :] * freqs[:, None]   # [32, S]
    cos2 = np.concatenate([np.cos(theta), np.cos(theta)], 0).astype(f32)
    sinpm = np.concatenate([-np.sin(theta), np.sin(theta)], 0).astype(f32)
    p = np.arange(P)[:, None, None]
    rr = np.arange(4)[None, :, None]
    f = np.arange(512)[None, None, :]
    maskadd = np.where(p <= f - P * rr, 0.0, NEG).astype(f32).reshape(P, 4 * 512)

    in_maps = []
    for c in range(8):
        b, g = divmod(c, 4)
        cs, ce = g * GCOL, (g + 1) * GCOL          # head cols of this group
        wUQ_g = _lhsT_layout(WUQf[:, cs:ce]).astype(bf16).reshape(HPG, P, KE * P)
        qr = WQRf[:, g * HPG * R:(g + 1) * HPG * R]
        wQR_g = _lhsT_layout(_rope_perm_cols(qr)).astype(bf16).reshape(2, P, KE * P)
        wUK_g = _lhsT_layout(W_UK[:, cs:ce]).astype(bf16).reshape(HPG, P, KL * P)
        wUV_g = _rhs_layout(W_UV[:, cs:ce]).astype(bf16).reshape(P, KL * GCOL)
        wO_g = _rhs_layout(W_O[cs:ce, :]).astype(bf16).reshape(P, HPG * E)
        in_maps.append({
            "xT": np.ascontiguousarray(x[b].T).astype(bf16),
            "wUQ": wUQ_g, "wQR": wQR_g, "wDKV": wDKV_t,
            "wUK": wUK_g, "wUV": wUV_g, "wKR": wKR_t, "wO": wO_g,
            "cosq": cos2, "sinq": sinpm, "maskin": maskadd,
        })
    return in_maps


def _assemble(results):
    out = np.empty((B, S, E), np.float32)
    for b in range(B):
        acc = results[4 * b]["outT"].astype(np.float32).copy()
        for g in range(1, 4):
            acc += results[4 * b + g]["outT"]
        out[b] = acc.T
    return out


def kernel(**inputs):
    inputs = {k: np.asarray(v) for k, v in inputs.items()}
    if "nc" not in _CACHE:
        _CACHE["nc"] = build_nc()
    nc = _CACHE["nc"]
    in_maps = _host_inputs(inputs)
    res = run_bass_kernel_spmd(nc, in_maps, core_ids=list(range(8)))
    return _assemble(res.results)


# revision 40
# speedup vs baseline: 1.0079x; 1.0079x over previous
"""Multi-Head Latent Attention for Trainium2, sharded over 8 NeuronCores.

Sharding: batch (2) x head-groups (4 of 4 heads each) -> 8 cores.
Host folds the shared down-projection into the per-group query weights
(W_UQ' = W_DQ @ W_UQ[:,g], W_QR' = W_DQ @ W_QR[:,g]), so each core runs:
    phase 1 (one pass over x^T):  c_KV^T, k_rope^T, q_C^T, q_rope^T
    phase 2+3 (per 512-seq block): k_C^T, v_C, then causal flash
        attention on transposed scores [keys, queries]
    phase 4: partial out^T = W_O[g]^T attnout^T
Host sums the 4 head-group partials per batch and transposes back.

All matmuls run in bf16 (1 PE cycle/row, any N) with f32 PSUM
accumulation; every intermediate stays SBUF-resident (no DRAM scratch).
Softmax denominators accumulate in PSUM via a ones-column matmul; the
reciprocal is broadcast across partitions with a K=1 ones-row matmul
instead of a DRAM bounce.
"""
import numpy as np

import concourse.bass as bass
import concourse.mybir as mybir
import concourse.tile as tile
from concourse import bacc
from concourse.bass_utils import run_bass_kernel_spmd

F32 = mybir.dt.float32
BF16 = mybir.dt.bfloat16
Exp = mybir.ActivationFunctionType.Exp
Copy = mybir.ActivationFunctionType.Copy
Mult = mybir.AluOpType.mult

B, S, E = 2, 2048, 2048
H = 16
DH = 128
LOW = 512
R = 64
BASE = 10000.0
HPG = 4               # heads per group (per core)
GCOL = HPG * DH       # 512 columns of this group's heads
P = 128
KE = E // P           # 16 k-tiles over E
KL = LOW // P         # 4 k-tiles over LOW
ST = S // P           # 16 seq tiles of 128
SBN = S // 512        # 4 seq blocks of 512
NEG = -3.0e38
SCALE = 1.0 / float(np.sqrt(DH + R))

_CACHE = {}


def _lhsT_layout(w):
    """[K, M] -> [MT, 128, KT, 128] so slice [mo] is an SBUF tile
    [128p, KT, 128m] with element [p, ko, m] = w[ko*128+p, mo*128+m]."""
    K, M = w.shape
    return np.ascontiguousarray(
        w.reshape(K // P, P, M // P, P).transpose(2, 1, 0, 3))


def _rhs_layout(w):
    """[K, N] -> [128, KT, N]: element [p, ko, n] = w[ko*128+p, n]."""
    K, N = w.shape
    return np.ascontiguousarray(w.reshape(K // P, P, N).transpose(1, 0, 2))


def _rope_perm_cols(w, rope_dim=R):
    """Permute each rope_dim-column block to [evens, odds] order."""
    K, M = w.shape
    nh = M // rope_dim
    w = w.reshape(K, nh, rope_dim)
    perm = np.concatenate([np.arange(0, rope_dim, 2), np.arange(1, rope_dim, 2)])
    return np.ascontiguousarray(w[:, :, perm].reshape(K, M))


def build_nc():
    nc = bacc.Bacc("TRN2", target_bir_lowering=False, debug=False, num_devices=8)

    xT = nc.dram_tensor("xT", [E, S], BF16, kind="ExternalInput")
    wUQ = nc.dram_tensor("wUQ", [HPG, P, KE * P], BF16, kind="ExternalInput")
    wQR = nc.dram_tensor("wQR", [2, P, KE * P], BF16, kind="ExternalInput")
    wDKV = nc.dram_tensor("wDKV", [KL, P, KE * P], BF16, kind="ExternalInput")
    wUK = nc.dram_tensor("wUK", [HPG, P, KL * P], BF16, kind="ExternalInput")
    wUV = nc.dram_tensor("wUV", [P, KL * GCOL], BF16, kind="ExternalInput")
    wKR = nc.dram_tensor("wKR", [P, KE * R], BF16, kind="ExternalInput")
    wO = nc.dram_tensor("wO", [P, HPG * E], BF16, kind="ExternalInput")
    cosq = nc.dram_tensor("cosq", [R, S], F32, kind="ExternalInput")   # [cos;cos]
    sinq = nc.dram_tensor("sinq", [R, S], F32, kind="ExternalInput")   # [-sin;sin]
    maskin = nc.dram_tensor("maskin", [P, 4 * 512], F32, kind="ExternalInput")

    outT = nc.dram_tensor("outT", [E, S], BF16, kind="ExternalOutput")

    with tile.TileContext(nc) as tc:
        with tc.tile_pool(name="persist", bufs=1) as persist:
            kropeT = persist.tile([R, S], BF16, tag="kropeT")
            t_ones = persist.tile([P, 1], BF16, tag="ones")
            nc.vector.memset(t_ones, 1.0)
            t_onesr = persist.tile([1, P], BF16, tag="onesr")
            nc.vector.memset(t_onesr, 1.0)
            qCT = persist.tile([P, HPG, S], BF16, tag="qCT")
            qrE = persist.tile([R, 2, S], BF16, tag="qrE")   # heads 0,2
            qrO = persist.tile([R, 2, S], BF16, tag="qrO")   # heads 1,3
            cKV = persist.tile([P, KL, S], BF16, tag="cKV")
            kCT = persist.tile([P, HPG, S], BF16, tag="kCT")
            vC = persist.tile([P, ST, GCOL], BF16, tag="vC")

            # phase-2/3/4 fixed weights: pool opened before phase 1 so their
            # DMAs can fire mid-phase-1 (once the x DMA queue has drained)
            stack_wfix2 = tc.tile_pool(name="wfix2", bufs=1)
            wfix2 = stack_wfix2.__enter__()
            t_wuk = [wfix2.tile([P, KL * P], BF16, tag=f"wuk{mo}",
                                name=f"wuk{mo}") for mo in range(HPG)]
            t_wuv = wfix2.tile([P, KL * GCOL], BF16, tag="wuv")
            t_wo = wfix2.tile([P, HPG * E], BF16, tag="wo")

            def fire_wukv_dmas():
                for mo in range(HPG):
                    nc.sync.dma_start(out=t_wuk[mo], in_=wUK[mo])
                nc.sync.dma_start(out=t_wuv, in_=wUV[:, :])

            def fire_wo_dma():
                nc.sync.dma_start(out=t_wo, in_=wO[:, :])

            with tc.tile_pool(name="tabp", bufs=1) as tabp:
                t_cos = tabp.tile([R, S], F32, tag="cos")
                t_sin = tabp.tile([R, S], F32, tag="sin")

                def rope_from_psum(pool, psum, base, scol, dst):
                    """dst[...] (64 x 512, bf16) = rope(psum[base:base+64]).

                    psum rows [base:base+32]=x1, [base+32:base+64]=x2 (host
                    permuted weight cols). dst = x*[c;c] + swap(x)*[-s;s].
                    TensorCopy may shift partitions; TensorTensor operands
                    must share a base partition, hence copy-then-mul."""
                    sl = slice(scol, scol + 512)
                    swp = pool.tile([R, 512], F32, tag="swp")
                    nc.vector.tensor_copy(out=swp[0:32, :],
                                          in_=psum[base + 32:base + R, :])
                    nc.vector.tensor_copy(out=swp[32:R, :],
                                          in_=psum[base:base + 32, :])
                    nc.vector.tensor_mul(out=swp, in0=swp, in1=t_sin[:, sl])
                    aln = pool.tile([R, 512], F32, tag="aln")
                    if base == 0:
                        nc.vector.tensor_mul(out=aln, in0=psum[0:R, :],
                                             in1=t_cos[:, sl])
                    else:
                        nc.vector.tensor_copy(out=aln,
                                              in_=psum[base:base + R, :])
                        nc.vector.tensor_mul(out=aln, in0=aln,
                                             in1=t_cos[:, sl])
                    nc.vector.tensor_add(out=dst, in0=aln, in1=swp)

                # ---- Phase 1: one pass over x^T -> c_KV^T, k_rope^T,
                #      q_C^T, q_rope^T (all SBUF-resident) ----
                with (
                    tc.tile_pool(name="xt", bufs=7) as xtp,
                    tc.tile_pool(name="wfix1", bufs=1) as wfix1,
                    tc.tile_pool(name="ropetmp", bufs=1) as ropetmp,
                    tc.tile_pool(name="ps_p1", bufs=6, space="PSUM") as ps_p1,
                    tc.tile_pool(name="ps_kr", bufs=1, space="PSUM") as ps_kr,
                ):
                    # DMA issue order tracks first use: wDKV[0], then the
                    # first x block, then the remaining weights.
                    t_wdkv = [wfix1.tile([P, KE * P], BF16, tag=f"wdkv{mo}",
                                         name=f"wdkv{mo}") for mo in range(KL)]
                    t_wkr = wfix1.tile([P, KE * R], BF16, tag="wkr")
                    t_wuq = [wfix1.tile([P, KE * P], BF16, tag=f"wuq{mo}",
                                        name=f"wuq{mo}") for mo in range(HPG)]
                    t_wqr = [wfix1.tile([P, KE * P], BF16, tag=f"wqr{mo}",
                                        name=f"wqr{mo}") for mo in range(2)]
                    nc.sync.dma_start(out=t_wqr[0], in_=wQR[0])

                    for sb in range(SBN):
                        ssl = slice(sb * 512, (sb + 1) * 512)
                        # 4 tiles per DMA: 4x fewer serialized HWDGE slots
                        xts = []
                        for g in range(4):
                            t = xtp.tile([P, 4, 512], BF16, tag="xt")
                            src0 = xT[g * 4 * P:g * 4 * P + P, ssl]
                            nc.sync.dma_start(out=t, in_=bass.AP(
                                tensor=src0.tensor, offset=src0.offset,
                                ap=[[S, P], [P * S, 4], [1, 512]]))
                            xts.extend(t[:, j, :] for j in range(4))
                        if sb == 0:
                            nc.sync.dma_start(out=t_wqr[1], in_=wQR[1])
                            nc.sync.dma_start(out=t_wkr, in_=wKR[:, :])
                            for mo in range(KL):
                                nc.sync.dma_start(out=t_wdkv[mo], in_=wDKV[mo])
                            for mo in range(HPG):
                                nc.sync.dma_start(out=t_wuq[mo], in_=wUQ[mo])
                            # rope tables gate only DVE work (slack there)
                            nc.sync.dma_start(out=t_cos, in_=cosq[:, :])
                            nc.sync.dma_start(out=t_sin, in_=sinq[:, :])
                            fire_wukv_dmas()
                        elif sb == 2:
                            fire_wo_dma()

                        for mo in range(2):    # q_rope^T (2 heads per chain)
                            psum = ps_p1.tile([P, 512], F32, tag="p")
                            for k in range(KE):
                                nc.tensor.matmul(
                                    psum, t_wqr[mo][:, k * P:(k + 1) * P], xts[k],
                                    start=(k == 0), stop=(k == KE - 1))
                            rope_from_psum(ropetmp, psum, 0, sb * 512,
                                           qrE[:, mo, ssl])
                            rope_from_psum(ropetmp, psum, R, sb * 512,
                                           qrO[:, mo, ssl])
                        # k_rope^T
                        psum = ps_kr.tile([R, 512], F32, tag="pkr")
                        for k in range(KE):
                            nc.tensor.matmul(
                                psum, t_wkr[:, k * R:(k + 1) * R], xts[k],
                                start=(k == 0), stop=(k == KE - 1))
                        rope_from_psum(ropetmp, psum, 0, sb * 512,
                                       kropeT[:, ssl])
                        for mo in range(KL):   # c_KV^T
                            psum = ps_p1.tile([P, 512], F32, tag="p")
                            for k in range(KE):
                                nc.tensor.matmul(
                                    psum, t_wdkv[mo][:, k * P:(k + 1) * P], xts[k],
                                    start=(k == 0), stop=(k == KE - 1))
                            nc.scalar.activation(out=cKV[:, mo, ssl],
                                                 in_=psum, func=Copy)
                        for mo in range(HPG):  # q_C^T
                            psum = ps_p1.tile([P, 512], F32, tag="p")
                            for k in range(KE):
                                nc.tensor.matmul(
                                    psum, t_wuq[mo][:, k * P:(k + 1) * P], xts[k],
                                    start=(k == 0), stop=(k == KE - 1))
                            if mo % 2 == 0:
                                nc.scalar.activation(out=qCT[:, mo, ssl],
                                                     in_=psum, func=Copy)
                            else:
                                nc.vector.tensor_copy(out=qCT[:, mo, ssl],
                                                      in_=psum)
                        for mo in range(HPG):  # k_C^T
                            psum = ps_p1.tile([P, 512], F32, tag="p")
                            for k in range(KL):
                                nc.tensor.matmul(
                                    psum, t_wuk[mo][:, k * P:(k + 1) * P],
                                    cKV[:, k, ssl],
                                    start=(k == 0), stop=(k == KL - 1))
                            if mo % 2 == 0:
                                nc.scalar.activation(out=kCT[:, mo, ssl],
                                                     in_=psum, func=Copy)
                            else:
                                nc.vector.tensor_copy(out=kCT[:, mo, ssl],
                                                      in_=psum)
                        for loc in range(4):   # v_C
                            st = sb * 4 + loc
                            psum = ps_p1.tile([P, GCOL], F32, tag="p")
                            for k in range(KL):
                                nc.tensor.matmul(
                                    psum, cKV[:, k, st * P:(st + 1) * P],
                                    t_wuv[:, k * GCOL:(k + 1) * GCOL],
                                    start=(k == 0), stop=(k == KL - 1))
                            if loc % 2 == 0:
                                nc.scalar.activation(out=vC[:, st, :],
                                                     in_=psum, func=Copy)
                            else:
                                nc.vector.tensor_copy(out=vC[:, st, :],
                                                      in_=psum)

            # ---- Phases 2+3 per seq block: k_C^T, v_C, then attention ----
            stack_p2 = tc.tile_pool(name="persist2", bufs=1)
            persist2 = stack_p2.__enter__()
            aoT = persist2.tile([P, HPG, S], BF16, tag="aoT")
            with (
                tc.tile_pool(name="wfix3", bufs=1) as wfix3,
                tc.tile_pool(name="att", bufs=4) as att,
                tc.tile_pool(name="attsm", bufs=3) as attsm,
                tc.tile_pool(name="ps_s", bufs=4, space="PSUM") as ps_s,
                tc.tile_pool(name="ps_o", bufs=2, space="PSUM") as ps_o,
                tc.tile_pool(name="ps_d", bufs=2, space="PSUM") as ps_d,
            ):
                t_mask = wfix3.tile([P, 4 * 512], F32, tag="mask")
                nc.sync.dma_start(out=t_mask, in_=maskin[:, :])

                # normalization of head block (h, ssl) is deferred until the
                # next head's score loop has been issued, hiding the
                # recip->copy->broadcast-matmul latency under PE work
                pend = []

                def normalize_pending():
                    h, pssl, psum_o, psum_d = pend.pop()
                    rec32 = attsm.tile([1, 512], F32, tag="rec32")
                    nc.vector.reciprocal(out=rec32, in_=psum_d)
                    rec16 = attsm.tile([1, 512], BF16, tag="rec16")
                    nc.scalar.activation(out=rec16, in_=rec32, func=Copy)
                    psum_bc = ps_s.tile([P, 512], F32, tag="p")
                    nc.tensor.matmul(psum_bc, t_onesr, rec16,
                                     start=True, stop=True)
                    bc16 = attsm.tile([P, 512], BF16, tag="bc16")
                    nc.scalar.activation(out=bc16, in_=psum_bc, func=Copy)
                    nc.vector.tensor_tensor(aoT[:, h, pssl], psum_o,
                                            bc16, Mult)

                for sb in range(SBN):
                    ssl = slice(sb * 512, (sb + 1) * 512)
                    # attention for query block sb over key tiles 0..4sb+3
                    T = 4 * (sb + 1)
                    for h in range(HPG):
                        psum_o = ps_o.tile([P, 512], F32, tag="p")
                        psum_d = ps_d.tile([1, 512], F32, tag="p")
                        for tt in range(T):
                            tsl = slice(tt * P, (tt + 1) * P)
                            r = tt - 4 * sb
                            # diagonal key tiles: queries < 128r of this
                            # block are fully masked -> trim the free dim
                            qo = P * r if r > 0 else 0
                            qs = slice(qo, 512)
                            gqs = slice(sb * 512 + qo, (sb + 1) * 512)
                            hqs = (qrE if h % 2 == 0 else qrO)[:, h // 2, gqs]
                            sg = qo > 0
                            psum_s = ps_s.tile([P, 512], F32, tag="p")
                            nc.tensor.matmul(psum_s[:, qs], kCT[:, h, tsl],
                                             qCT[:, h, gqs],
                                             start=True, stop=False)
                            nc.tensor.matmul(psum_s[:, qs], kropeT[:, tsl],
                                             hqs, start=False, stop=True)
                            expT = att.tile([P, 512], BF16, tag="expT")
                            if r >= 0:
                                masked = attsm.tile([P, 512], BF16,
                                                    tag="masked")
                                nc.vector.tensor_add(
                                    out=masked[:, qs], in0=psum_s[:, qs],
                                    in1=t_mask[:, r * 512 + qo:(r + 1) * 512])
                                nc.scalar.activation(out=expT[:, qs],
                                                     in_=masked[:, qs],
                                                     func=Exp)
                            else:
                                nc.scalar.activation(out=expT, in_=psum_s,
                                                     func=Exp)
                            nc.tensor.matmul(psum_d[:, qs], t_ones,
                                             expT[:, qs],
                                             start=(tt == 0), stop=(tt == T - 1),
                                             skip_group_check=sg)
                            nc.tensor.matmul(psum_o[:, qs],
                                             vC[:, tt, h * DH:(h + 1) * DH],
                                             expT[:, qs],
                                             start=(tt == 0), stop=(tt == T - 1),
                                             skip_group_check=sg)
                        if pend:
                            normalize_pending()
                        pend.append((h, ssl, psum_o, psum_d))
                if pend:
                    normalize_pending()

            # ---- Phase 4: out^T partial = W_O[g]^T @ aoT ----
            with (
                tc.tile_pool(name="oout", bufs=6) as oout,
                tc.tile_pool(name="ps_w", bufs=4, space="PSUM") as ps_w,
            ):
                for sb in range(SBN):
                    ssl = slice(sb * 512, (sb + 1) * 512)
                    for mo in range(KE):
                        psum_w = ps_w.tile([P, 512], F32, tag="p")
                        for k in range(HPG):
                            nc.tensor.matmul(psum_w,
                                             t_wo[:, k * E + mo * P:k * E + (mo + 1) * P],
                                             aoT[:, k, ssl],
                                             start=(k == 0), stop=(k == HPG - 1))
                        ot = oout.tile([P, 512], BF16, tag="oout")
                        if mo % 2 == 0:
                            nc.scalar.activation(out=ot, in_=psum_w, func=Copy)
                        else:
                            nc.vector.tensor_copy(out=ot, in_=psum_w)
                        nc.sync.dma_start(
                            out=outT[mo * P:(mo + 1) * P, ssl], in_=ot)
            stack_p2.__exit__(None, None, None)
            stack_wfix2.__exit__(None, None, None)

    nc.compile()
    return nc


def _host_inputs(inputs):
    """Per-core input maps (host-side sharding + weight folding/pre-tiling)."""
    f32 = np.float32
    bf16 = mybir.dt.np(BF16)
    x = inputs["x"]
    W_DQ = inputs["W_DQ"].astype(f32)
    W_UQ, W_QR = inputs["W_UQ"].astype(f32), inputs["W_QR"].astype(f32)
    W_DKV, W_UK = inputs["W_DKV"].astype(f32), inputs["W_UK"].astype(f32)
    W_KR, W_UV = inputs["W_KR"].astype(f32), inputs["W_UV"].astype(f32)
    W_O = inputs["W_O"].astype(f32)

    # fold the shared down-projection into the query weights
    WUQf = (W_DQ @ W_UQ) * SCALE          # [E, E]
    WQRf = (W_DQ @ W_QR) * SCALE          # [E, R*H]

    # shared across cores
    wDKV_t = _lhsT_layout(W_DKV).astype(bf16).reshape(KL, P, KE * P)
    wKR_t = _rhs_layout(_rope_perm_cols(W_KR)).astype(bf16).reshape(P, KE * R)
    half = R // 2
    freqs = BASE ** (-np.arange(half, dtype=np.float64) / half)
    theta = np.arange(S, dtype=np.float64)[None, :] * freqs[:, None]   # [32, S]
    cos2 = np.concatenate([np.cos(theta), np.cos(theta)], 0).astype(f32)
    sinpm = np.concatenate([-np.sin(theta), np.sin(theta)], 0).astype(f32)
    p = np.arange(P)[:, None, None]
    rr = np.arange(4)[None, :, None]
    f = np.arange(512)[None, None, :]
    maskadd = np.where(p <= f - P * rr, 0.0, NEG).astype(f32).reshape(P, 4 * 512)

    in_maps = []
    for c in range(8):
        b, g = divmod(c, 4)
        cs, ce = g * GCOL, (g + 1) * GCOL          # head cols of this group
        wUQ_g = _lhsT_layout(WUQf[:, cs:ce]).astype(bf16).reshape(HPG, P, KE * P)
        qr = WQRf[:, g * HPG * R:(g + 1) * HPG * R]
        wQR_g = _lhsT_layout(_rope_perm_cols(qr)).astype(bf16).reshape(2, P, KE * P)
        wUK_g = _lhsT_layout(W_UK[:, cs:ce]).astype(bf16).reshape(HPG, P, KL * P)
        wUV_g = _rhs_layout(W_UV[:, cs:ce]).astype(bf16).reshape(P, KL * GCOL)
        wO_g = _rhs_layout(W_O[cs:ce, :]).astype(bf16).reshape(P, HPG * E)
        in_maps.append({
            "xT": np.ascontiguousarray(x[b].T).astype(bf16),
            "wUQ": wUQ_g, "wQR": wQR_g, "wDKV": wDKV_t,
            "wUK": wUK_g, "wUV": wUV_g, "wKR": wKR_t, "wO": wO_g,
            "cosq": cos2, "sinq": sinpm, "maskin": maskadd,
        })
    return in_maps


def _assemble(results):
    out = np.empty((B, S, E), np.float32)
    for b in range(B):
        acc = results[4 * b]["outT"].astype(np.float32).copy()
        for g in range(1, 4):
            acc += results[4 * b + g]["outT"]
        out[b] = acc.T
    return out


def kernel(**inputs):
    inputs = {k: np.asarray(v) for k, v in inputs.items()}
    if "nc" not in _CACHE:
        _CACHE["nc"] = build_nc()
    nc = _CACHE["nc"]
    in_maps = _host_inputs(inputs)
    res = run_bass_kernel_spmd(nc, in_maps, core_ids=list(range(8)))
    return _assemble(res.results)


# revision 41
# speedup vs baseline: 1.0175x; 1.0096x over previous
"""Multi-Head Latent Attention for Trainium2, sharded over 8 NeuronCores.

Sharding: batch (2) x head-groups (4 of 4 heads each) -> 8 cores.
Host folds the shared down-projection into the per-group query weights
(W_UQ' = W_DQ @ W_UQ[:,g], W_QR' = W_DQ @ W_QR[:,g]), so each core runs:
    phase 1 (one pass over x^T):  c_KV^T, k_rope^T, q_C^T, q_rope^T
    phase 2+3 (per 512-seq block): k_C^T, v_C, then causal flash
        attention on transposed scores [keys, queries]
    phase 4: partial out^T = W_O[g]^T attnout^T
Host sums the 4 head-group partials per batch and transposes back.

All matmuls run in bf16 (1 PE cycle/row, any N) with f32 PSUM
accumulation; every intermediate stays SBUF-resident (no DRAM scratch).
Softmax denominators accumulate in PSUM via a ones-column matmul; the
reciprocal is broadcast across partitions with a K=1 ones-row matmul
instead of a DRAM bounce.
"""
import numpy as np

import concourse.bass as bass
import concourse.mybir as mybir
import concourse.tile as tile
from concourse import bacc
from concourse.bass_utils import run_bass_kernel_spmd

F32 = mybir.dt.float32
BF16 = mybir.dt.bfloat16
Exp = mybir.ActivationFunctionType.Exp
Copy = mybir.ActivationFunctionType.Copy
Mult = mybir.AluOpType.mult

B, S, E = 2, 2048, 2048
H = 16
DH = 128
LOW = 512
R = 64
BASE = 10000.0
HPG = 4               # heads per group (per core)
GCOL = HPG * DH       # 512 columns of this group's heads
P = 128
KE = E // P           # 16 k-tiles over E
KL = LOW // P         # 4 k-tiles over LOW
ST = S // P           # 16 seq tiles of 128
SBN = S // 512        # 4 seq blocks of 512
NEG = -3.0e38
SCALE = 1.0 / float(np.sqrt(DH + R))

_CACHE = {}


def _lhsT_layout(w):
    """[K, M] -> [MT, 128, KT, 128] so slice [mo] is an SBUF tile
    [128p, KT, 128m] with element [p, ko, m] = w[ko*128+p, mo*128+m]."""
    K, M = w.shape
    return np.ascontiguousarray(
        w.reshape(K // P, P, M // P, P).transpose(2, 1, 0, 3))


def _rhs_layout(w):
    """[K, N] -> [128, KT, N]: element [p, ko, n] = w[ko*128+p, n]."""
    K, N = w.shape
    return np.ascontiguousarray(w.reshape(K // P, P, N).transpose(1, 0, 2))


def _rope_perm_cols(w, rope_dim=R):
    """Permute each rope_dim-column block to [evens, odds] order."""
    K, M = w.shape
    nh = M // rope_dim
    w = w.reshape(K, nh, rope_dim)
    perm = np.concatenate([np.arange(0, rope_dim, 2), np.arange(1, rope_dim, 2)])
    return np.ascontiguousarray(w[:, :, perm].reshape(K, M))


def build_nc():
    nc = bacc.Bacc("TRN2", target_bir_lowering=False, debug=False, num_devices=8)

    xT = nc.dram_tensor("xT", [E, S], BF16, kind="ExternalInput")
    wUQ = nc.dram_tensor("wUQ", [HPG, P, KE * P], BF16, kind="ExternalInput")
    wQR = nc.dram_tensor("wQR", [2, P, KE * P], BF16, kind="ExternalInput")
    wDKV = nc.dram_tensor("wDKV", [KL, P, KE * P], BF16, kind="ExternalInput")
    wUK = nc.dram_tensor("wUK", [HPG, P, KL * P], BF16, kind="ExternalInput")
    wUV = nc.dram_tensor("wUV", [P, KL * GCOL], BF16, kind="ExternalInput")
    wKR = nc.dram_tensor("wKR", [P, KE * R], BF16, kind="ExternalInput")
    wO = nc.dram_tensor("wO", [P, HPG * E], BF16, kind="ExternalInput")
    cosq = nc.dram_tensor("cosq", [R, S], F32, kind="ExternalInput")   # [cos;cos]
    sinq = nc.dram_tensor("sinq", [R, S], F32, kind="ExternalInput")   # [-sin;sin]
    maskin = nc.dram_tensor("maskin", [P, 4 * 512], F32, kind="ExternalInput")

    outT = nc.dram_tensor("outT", [E, S], BF16, kind="ExternalOutput")
    # DRAM scratch for the tail softmax-normalize broadcasts (must be an
    # ExternalOutput in this environment)
    dscr = nc.dram_tensor("dscr", [8, 512], F32, kind="ExternalOutput")

    with tile.TileContext(nc) as tc:
        with tc.tile_pool(name="persist", bufs=1) as persist:
            kropeT = persist.tile([R, S], BF16, tag="kropeT")
            t_ones = persist.tile([P, 1], BF16, tag="ones")
            nc.vector.memset(t_ones, 1.0)
            t_onesr = persist.tile([1, P], BF16, tag="onesr")
            nc.vector.memset(t_onesr, 1.0)
            qCT = persist.tile([P, HPG, S], BF16, tag="qCT")
            qrE = persist.tile([R, 2, S], BF16, tag="qrE")   # heads 0,2
            qrO = persist.tile([R, 2, S], BF16, tag="qrO")   # heads 1,3
            cKV = persist.tile([P, KL, S], BF16, tag="cKV")
            kCT = persist.tile([P, HPG, S], BF16, tag="kCT")
            vC = persist.tile([P, ST, GCOL], BF16, tag="vC")

            # phase-2/3/4 fixed weights: pool opened before phase 1 so their
            # DMAs can fire mid-phase-1 (once the x DMA queue has drained)
            stack_wfix2 = tc.tile_pool(name="wfix2", bufs=1)
            wfix2 = stack_wfix2.__enter__()
            t_wuk = [wfix2.tile([P, KL * P], BF16, tag=f"wuk{mo}",
                                name=f"wuk{mo}") for mo in range(HPG)]
            t_wuv = wfix2.tile([P, KL * GCOL], BF16, tag="wuv")
            t_wo = wfix2.tile([P, HPG * E], BF16, tag="wo")

            def fire_wukv_dmas():
                for mo in range(HPG):
                    nc.sync.dma_start(out=t_wuk[mo], in_=wUK[mo])
                nc.sync.dma_start(out=t_wuv, in_=wUV[:, :])

            def fire_wo_dma():
                nc.sync.dma_start(out=t_wo, in_=wO[:, :])

            with tc.tile_pool(name="tabp", bufs=1) as tabp:
                t_cos = tabp.tile([R, S], F32, tag="cos")
                t_sin = tabp.tile([R, S], F32, tag="sin")

                def rope_from_psum(pool, psum, base, scol, dst):
                    """dst[...] (64 x 512, bf16) = rope(psum[base:base+64]).

                    psum rows [base:base+32]=x1, [base+32:base+64]=x2 (host
                    permuted weight cols). dst = x*[c;c] + swap(x)*[-s;s].
                    TensorCopy may shift partitions; TensorTensor operands
                    must share a base partition, hence copy-then-mul."""
                    sl = slice(scol, scol + 512)
                    swp = pool.tile([R, 512], F32, tag="swp")
                    nc.vector.tensor_copy(out=swp[0:32, :],
                                          in_=psum[base + 32:base + R, :])
                    nc.vector.tensor_copy(out=swp[32:R, :],
                                          in_=psum[base:base + 32, :])
                    nc.vector.tensor_mul(out=swp, in0=swp, in1=t_sin[:, sl])
                    aln = pool.tile([R, 512], F32, tag="aln")
                    if base == 0:
                        nc.vector.tensor_mul(out=aln, in0=psum[0:R, :],
                                             in1=t_cos[:, sl])
                    else:
                        nc.vector.tensor_copy(out=aln,
                                              in_=psum[base:base + R, :])
                        nc.vector.tensor_mul(out=aln, in0=aln,
                                             in1=t_cos[:, sl])
                    nc.vector.tensor_add(out=dst, in0=aln, in1=swp)

                # ---- Phase 1: one pass over x^T -> c_KV^T, k_rope^T,
                #      q_C^T, q_rope^T (all SBUF-resident) ----
                with (
                    tc.tile_pool(name="xt", bufs=7) as xtp,
                    tc.tile_pool(name="wfix1", bufs=1) as wfix1,
                    tc.tile_pool(name="ropetmp", bufs=1) as ropetmp,
                    tc.tile_pool(name="ps_p1", bufs=6, space="PSUM") as ps_p1,
                    tc.tile_pool(name="ps_kr", bufs=1, space="PSUM") as ps_kr,
                ):
                    # DMA issue order tracks first use: wDKV[0], then the
                    # first x block, then the remaining weights.
                    t_wdkv = [wfix1.tile([P, KE * P], BF16, tag=f"wdkv{mo}",
                                         name=f"wdkv{mo}") for mo in range(KL)]
                    t_wkr = wfix1.tile([P, KE * R], BF16, tag="wkr")
                    t_wuq = [wfix1.tile([P, KE * P], BF16, tag=f"wuq{mo}",
                                        name=f"wuq{mo}") for mo in range(HPG)]
                    t_wqr = [wfix1.tile([P, KE * P], BF16, tag=f"wqr{mo}",
                                        name=f"wqr{mo}") for mo in range(2)]
                    nc.sync.dma_start(out=t_wqr[0], in_=wQR[0])

                    for sb in range(SBN):
                        ssl = slice(sb * 512, (sb + 1) * 512)
                        # 4 tiles per DMA: 4x fewer serialized HWDGE slots
                        xts = []
                        for g in range(4):
                            t = xtp.tile([P, 4, 512], BF16, tag="xt")
                            src0 = xT[g * 4 * P:g * 4 * P + P, ssl]
                            nc.sync.dma_start(out=t, in_=bass.AP(
                                tensor=src0.tensor, offset=src0.offset,
                                ap=[[S, P], [P * S, 4], [1, 512]]))
                            xts.extend(t[:, j, :] for j in range(4))
                        if sb == 0:
                            nc.sync.dma_start(out=t_wqr[1], in_=wQR[1])
                            nc.sync.dma_start(out=t_wkr, in_=wKR[:, :])
                            for mo in range(KL):
                                nc.sync.dma_start(out=t_wdkv[mo], in_=wDKV[mo])
                            for mo in range(HPG):
                                nc.sync.dma_start(out=t_wuq[mo], in_=wUQ[mo])
                            # rope tables gate only DVE work (slack there)
                            nc.sync.dma_start(out=t_cos, in_=cosq[:, :])
                            nc.sync.dma_start(out=t_sin, in_=sinq[:, :])
                            fire_wukv_dmas()
                        elif sb == 2:
                            fire_wo_dma()

                        for mo in range(2):    # q_rope^T (2 heads per chain)
                            psum = ps_p1.tile([P, 512], F32, tag="p")
                            for k in range(KE):
                                nc.tensor.matmul(
                                    psum, t_wqr[mo][:, k * P:(k + 1) * P], xts[k],
                                    start=(k == 0), stop=(k == KE - 1))
                            rope_from_psum(ropetmp, psum, 0, sb * 512,
                                           qrE[:, mo, ssl])
                            rope_from_psum(ropetmp, psum, R, sb * 512,
                                           qrO[:, mo, ssl])
                        # k_rope^T
                        psum = ps_kr.tile([R, 512], F32, tag="pkr")
                        for k in range(KE):
                            nc.tensor.matmul(
                                psum, t_wkr[:, k * R:(k + 1) * R], xts[k],
                                start=(k == 0), stop=(k == KE - 1))
                        rope_from_psum(ropetmp, psum, 0, sb * 512,
                                       kropeT[:, ssl])
                        for mo in range(KL):   # c_KV^T
                            psum = ps_p1.tile([P, 512], F32, tag="p")
                            for k in range(KE):
                                nc.tensor.matmul(
                                    psum, t_wdkv[mo][:, k * P:(k + 1) * P], xts[k],
                                    start=(k == 0), stop=(k == KE - 1))
                            nc.scalar.activation(out=cKV[:, mo, ssl],
                                                 in_=psum, func=Copy)
                        for mo in range(HPG):  # q_C^T
                            psum = ps_p1.tile([P, 512], F32, tag="p")
                            for k in range(KE):
                                nc.tensor.matmul(
                                    psum, t_wuq[mo][:, k * P:(k + 1) * P], xts[k],
                                    start=(k == 0), stop=(k == KE - 1))
                            if mo % 2 == 0:
                                nc.scalar.activation(out=qCT[:, mo, ssl],
                                                     in_=psum, func=Copy)
                            else:
                                nc.vector.tensor_copy(out=qCT[:, mo, ssl],
                                                      in_=psum)
                        for mo in range(HPG):  # k_C^T
                            psum = ps_p1.tile([P, 512], F32, tag="p")
                            for k in range(KL):
                                nc.tensor.matmul(
                                    psum, t_wuk[mo][:, k * P:(k + 1) * P],
                                    cKV[:, k, ssl],
                                    start=(k == 0), stop=(k == KL - 1))
                            if mo % 2 == 0:
                                nc.scalar.activation(out=kCT[:, mo, ssl],
                                                     in_=psum, func=Copy)
                            else:
                                nc.vector.tensor_copy(out=kCT[:, mo, ssl],
                                                      in_=psum)
                        for loc in range(4):   # v_C
                            st = sb * 4 + loc
                            psum = ps_p1.tile([P, GCOL], F32, tag="p")
                            for k in range(KL):
                                nc.tensor.matmul(
                                    psum, cKV[:, k, st * P:(st + 1) * P],
                                    t_wuv[:, k * GCOL:(k + 1) * GCOL],
                                    start=(k == 0), stop=(k == KL - 1))
                            if loc % 2 == 0:
                                nc.scalar.activation(out=vC[:, st, :],
                                                     in_=psum, func=Copy)
                            else:
                                nc.vector.tensor_copy(out=vC[:, st, :],
                                                      in_=psum)

            # ---- Phases 2+3 per seq block: k_C^T, v_C, then attention ----
            stack_p2 = tc.tile_pool(name="persist2", bufs=1)
            persist2 = stack_p2.__enter__()
            aoT = persist2.tile([P, HPG, S], BF16, tag="aoT")
            with (
                tc.tile_pool(name="wfix3", bufs=1) as wfix3,
                tc.tile_pool(name="att", bufs=4) as att,
                tc.tile_pool(name="attsm", bufs=3) as attsm,
                tc.tile_pool(name="ps_s", bufs=4, space="PSUM") as ps_s,
                tc.tile_pool(name="ps_o", bufs=2, space="PSUM") as ps_o,
                tc.tile_pool(name="ps_d", bufs=2, space="PSUM") as ps_d,
            ):
                t_mask = wfix3.tile([P, 4 * 512], F32, tag="mask")
                nc.sync.dma_start(out=t_mask, in_=maskin[:, :])

                # normalization of head block (h, ssl) is deferred until the
                # next head's score loop has been issued, hiding the
                # recip->copy->broadcast-matmul latency under PE work
                pend = []

                def normalize_pending(drow=None):
                    h, pssl, psum_o, psum_d = pend.pop()
                    if drow is None:
                        rec32 = attsm.tile([1, 512], F32, tag="rec32")
                        nc.vector.reciprocal(out=rec32, in_=psum_d)
                        rec16 = attsm.tile([1, 512], BF16, tag="rec16")
                        nc.scalar.activation(out=rec16, in_=rec32, func=Copy)
                        psum_bc = ps_s.tile([P, 512], F32, tag="p")
                        nc.tensor.matmul(psum_bc, t_onesr, rec16,
                                         start=True, stop=True)
                        bc16 = attsm.tile([P, 512], BF16, tag="bc16")
                        nc.scalar.activation(out=bc16, in_=psum_bc, func=Copy)
                        nc.vector.tensor_tensor(aoT[:, h, pssl], psum_o,
                                                bc16, Mult)
                        return
                    # tail blocks: no PE work follows to hide the broadcast
                    # matmul, so bounce 1/denom through DRAM on the idle
                    # Pool-engine DMA path instead (keeps the PE stream dry)
                    tmp4 = attsm.tile([4, 512], F32, tag="tmp4")
                    nc.vector.memset(tmp4, 0.0)
                    nc.vector.tensor_copy(out=tmp4[0:1, :], in_=psum_d)
                    nc.vector.reciprocal(out=tmp4[0:1, :], in_=tmp4[0:1, :])
                    nc.sync.dma_start(out=dscr[drow:drow + 4, :], in_=tmp4)
                    bc = attsm.tile([P, 512], F32, tag="bc")
                    srcr = dscr[drow, :]
                    nc.gpsimd.dma_start(out=bc, in_=bass.AP(
                        tensor=srcr.tensor, offset=srcr.offset,
                        ap=[[0, P]] + [list(x) for x in srcr.ap]))
                    nc.vector.tensor_tensor(aoT[:, h, pssl], psum_o, bc, Mult)

                for sb in range(SBN):
                    ssl = slice(sb * 512, (sb + 1) * 512)
                    # attention for query block sb over key tiles 0..4sb+3
                    T = 4 * (sb + 1)
                    for h in range(HPG):
                        psum_o = ps_o.tile([P, 512], F32, tag="p")
                        psum_d = ps_d.tile([1, 512], F32, tag="p")
                        for tt in range(T):
                            tsl = slice(tt * P, (tt + 1) * P)
                            r = tt - 4 * sb
                            # diagonal key tiles: queries < 128r of this
                            # block are fully masked -> trim the free dim
                            qo = P * r if r > 0 else 0
                            qs = slice(qo, 512)
                            gqs = slice(sb * 512 + qo, (sb + 1) * 512)
                            hqs = (qrE if h % 2 == 0 else qrO)[:, h // 2, gqs]
                            sg = qo > 0
                            psum_s = ps_s.tile([P, 512], F32, tag="p")
                            nc.tensor.matmul(psum_s[:, qs], kCT[:, h, tsl],
                                             qCT[:, h, gqs],
                                             start=True, stop=False)
                            nc.tensor.matmul(psum_s[:, qs], kropeT[:, tsl],
                                             hqs, start=False, stop=True)
                            expT = att.tile([P, 512], BF16, tag="expT")
                            if r >= 0:
                                masked = attsm.tile([P, 512], BF16,
                                                    tag="masked")
                                nc.vector.tensor_add(
                                    out=masked[:, qs], in0=psum_s[:, qs],
                                    in1=t_mask[:, r * 512 + qo:(r + 1) * 512])
                                nc.scalar.activation(out=expT[:, qs],
                                                     in_=masked[:, qs],
                                                     func=Exp)
                            else:
                                nc.scalar.activation(out=expT, in_=psum_s,
                                                     func=Exp)
                            nc.tensor.matmul(psum_d[:, qs], t_ones,
                                             expT[:, qs],
                                             start=(tt == 0), stop=(tt == T - 1),
                                             skip_group_check=sg)
                            nc.tensor.matmul(psum_o[:, qs],
                                             vC[:, tt, h * DH:(h + 1) * DH],
                                             expT[:, qs],
                                             start=(tt == 0), stop=(tt == T - 1),
                                             skip_group_check=sg)
                        if pend:
                            normalize_pending(
                                drow=0 if (sb == SBN - 1 and h == HPG - 1)
                                else None)
                        pend.append((h, ssl, psum_o, psum_d))
                if pend:
                    normalize_pending(drow=4)

            # ---- Phase 4: out^T partial = W_O[g]^T @ aoT ----
            with (
                tc.tile_pool(name="oout", bufs=6) as oout,
                tc.tile_pool(name="ps_w", bufs=4, space="PSUM") as ps_w,
            ):
                for sb in range(SBN):
                    ssl = slice(sb * 512, (sb + 1) * 512)
                    for mo in range(KE):
                        psum_w = ps_w.tile([P, 512], F32, tag="p")
                        for k in range(HPG):
                            nc.tensor.matmul(psum_w,
                                             t_wo[:, k * E + mo * P:k * E + (mo + 1) * P],
                                             aoT[:, k, ssl],
                                             start=(k == 0), stop=(k == HPG - 1))
                        ot = oout.tile([P, 512], BF16, tag="oout")
                        if mo % 2 == 0:
                            nc.scalar.activation(out=ot, in_=psum_w, func=Copy)
                        else:
                            nc.vector.tensor_copy(out=ot, in_=psum_w)
                        nc.sync.dma_start(
                            out=outT[mo * P:(mo + 1) * P, ssl], in_=ot)
            stack_p2.__exit__(None, None, None)
            stack_wfix2.__exit__(None, None, None)

    nc.compile()
    return nc


def _host_inputs(inputs):
    """Per-core input maps (host-side sharding + weight folding/pre-tiling)."""
    f32 = np.float32
    bf16 = mybir.dt.np(BF16)
    x = inputs["x"]
    W_DQ = inputs["W_DQ"].astype(f32)
    W_UQ, W_QR = inputs["W_UQ"].astype(f32), inputs["W_QR"].astype(f32)
    W_DKV, W_UK = inputs["W_DKV"].astype(f32), inputs["W_UK"].astype(f32)
    W_KR, W_UV = inputs["W_KR"].astype(f32), inputs["W_UV"].astype(f32)
    W_O = inputs["W_O"].astype(f32)

    # fold the shared down-projection into the query weights
    WUQf = (W_DQ @ W_UQ) * SCALE          # [E, E]
    WQRf = (W_DQ @ W_QR) * SCALE          # [E, R*H]

    # shared across cores
    wDKV_t = _lhsT_layout(W_DKV).astype(bf16).reshape(KL, P, KE * P)
    wKR_t = _rhs_layout(_rope_perm_cols(W_KR)).astype(bf16).reshape(P, KE * R)
    half = R // 2
    freqs = BASE ** (-np.arange(half, dtype=np.float64) / half)
    theta = np.arange(S, dtype=np.float64)[None, :] * freqs[:, None]   # [32, S]
    cos2 = np.concatenate([np.cos(theta), np.cos(theta)], 0).astype(f32)
    sinpm = np.concatenate([-np.sin(theta), np.sin(theta)], 0).astype(f32)
    p = np.arange(P)[:, None, None]
    rr = np.arange(4)[None, :, None]
    f = np.arange(512)[None, None, :]
    maskadd = np.where(p <= f - P * rr, 0.0, NEG).astype(f32).reshape(P, 4 * 512)

    in_maps = []
    for c in range(8):
        b, g = divmod(c, 4)
        cs, ce = g * GCOL, (g + 1) * GCOL          # head cols of this group
        wUQ_g = _lhsT_layout(WUQf[:, cs:ce]).astype(bf16).reshape(HPG, P, KE * P)
        qr = WQRf[:, g * HPG * R:(g + 1) * HPG * R]
        wQR_g = _lhsT_layout(_rope_perm_cols(qr)).astype(bf16).reshape(2, P, KE * P)
        wUK_g = _lhsT_layout(W_UK[:, cs:ce]).astype(bf16).reshape(HPG, P, KL * P)
        wUV_g = _rhs_layout(W_UV[:, cs:ce]).astype(bf16).reshape(P, KL * GCOL)
        wO_g = _rhs_layout(W_O[cs:ce, :]).astype(bf16).reshape(P, HPG * E)
        in_maps.append({
            "xT": np.ascontiguousarray(x[b].T).astype(bf16),
            "wUQ": wUQ_g, "wQR": wQR_g, "wDKV": wDKV_t,
            "wUK": wUK_g, "wUV": wUV_g, "wKR": wKR_t, "wO": wO_g,
            "cosq": cos2, "sinq": sinpm, "maskin": maskadd,
        })
    return in_maps


def _assemble(results):
    out = np.empty((B, S, E), np.float32)
    for b in range(B):
        acc = results[4 * b]["outT"].astype(np.float32).copy()
        for g in range(1, 4):
            acc += results[4 * b + g]["outT"]
        out[b] = acc.T
    return out


def kernel(**inputs):
    inputs = {k: np.asarray(v) for k, v in inputs.items()}
    if "nc" not in _CACHE:
        _CACHE["nc"] = build_nc()
    nc = _CACHE["nc"]
    in_maps = _host_inputs(inputs)
    res = run_bass_kernel_spmd(nc, in_maps, core_ids=list(range(8)))
    return _assemble(res.results)


# revision 44
# speedup vs baseline: 1.0184x; 1.0009x over previous
"""Multi-Head Latent Attention for Trainium2, sharded over 8 NeuronCores.

Sharding: batch (2) x head-groups (4 of 4 heads each) -> 8 cores.
Host folds the shared down-projection into the per-group query weights
(W_UQ' = W_DQ @ W_UQ[:,g], W_QR' = W_DQ @ W_QR[:,g]), so each core runs:
    phase 1 (one pass over x^T):  c_KV^T, k_rope^T, q_C^T, q_rope^T
    phase 2+3 (per 512-seq block): k_C^T, v_C, then causal flash
        attention on transposed scores [keys, queries]
    phase 4: partial out^T = W_O[g]^T attnout^T
Host sums the 4 head-group partials per batch and transposes back.

All matmuls run in bf16 (1 PE cycle/row, any N) with f32 PSUM
accumulation; every intermediate stays SBUF-resident (no DRAM scratch).
Softmax denominators accumulate in PSUM via a ones-column matmul; the
reciprocal is broadcast across partitions with a K=1 ones-row matmul
instead of a DRAM bounce.
"""
import numpy as np

import concourse.bass as bass
import concourse.mybir as mybir
import concourse.tile as tile
from concourse import bacc
from concourse.bass_utils import run_bass_kernel_spmd

F32 = mybir.dt.float32
BF16 = mybir.dt.bfloat16
Exp = mybir.ActivationFunctionType.Exp
Copy = mybir.ActivationFunctionType.Copy
Mult = mybir.AluOpType.mult

B, S, E = 2, 2048, 2048
H = 16
DH = 128
LOW = 512
R = 64
BASE = 10000.0
HPG = 4               # heads per group (per core)
GCOL = HPG * DH       # 512 columns of this group's heads
P = 128
KE = E // P           # 16 k-tiles over E
KL = LOW // P         # 4 k-tiles over LOW
ST = S // P           # 16 seq tiles of 128
SBN = S // 512        # 4 seq blocks of 512
NEG = -3.0e38
SCALE = 1.0 / float(np.sqrt(DH + R))

_CACHE = {}


def _lhsT_layout(w):
    """[K, M] -> [MT, 128, KT, 128] so slice [mo] is an SBUF tile
    [128p, KT, 128m] with element [p, ko, m] = w[ko*128+p, mo*128+m]."""
    K, M = w.shape
    return np.ascontiguousarray(
        w.reshape(K // P, P, M // P, P).transpose(2, 1, 0, 3))


def _rhs_layout(w):
    """[K, N] -> [128, KT, N]: element [p, ko, n] = w[ko*128+p, n]."""
    K, N = w.shape
    return np.ascontiguousarray(w.reshape(K // P, P, N).transpose(1, 0, 2))


def _rope_perm_cols(w, rope_dim=R):
    """Permute each rope_dim-column block to [evens, odds] order."""
    K, M = w.shape
    nh = M // rope_dim
    w = w.reshape(K, nh, rope_dim)
    perm = np.concatenate([np.arange(0, rope_dim, 2), np.arange(1, rope_dim, 2)])
    return np.ascontiguousarray(w[:, :, perm].reshape(K, M))


def build_nc():
    nc = bacc.Bacc("TRN2", target_bir_lowering=False, debug=False, num_devices=8)

    xT = nc.dram_tensor("xT", [E, S], BF16, kind="ExternalInput")
    wUQ = nc.dram_tensor("wUQ", [HPG, P, KE * P], BF16, kind="ExternalInput")
    wQR = nc.dram_tensor("wQR", [2, P, KE * P], BF16, kind="ExternalInput")
    wDKV = nc.dram_tensor("wDKV", [KL, P, KE * P], BF16, kind="ExternalInput")
    wUK = nc.dram_tensor("wUK", [HPG, P, KL * P], BF16, kind="ExternalInput")
    wUV = nc.dram_tensor("wUV", [P, KL * GCOL], BF16, kind="ExternalInput")
    wKR = nc.dram_tensor("wKR", [P, KE * R], BF16, kind="ExternalInput")
    wO = nc.dram_tensor("wO", [P, HPG * E], BF16, kind="ExternalInput")
    cosq = nc.dram_tensor("cosq", [R, S], F32, kind="ExternalInput")   # [cos;cos]
    sinq = nc.dram_tensor("sinq", [R, S], F32, kind="ExternalInput")   # [-sin;sin]
    maskin = nc.dram_tensor("maskin", [P, 4 * 512], F32, kind="ExternalInput")

    outT = nc.dram_tensor("outT", [E, S], BF16, kind="ExternalOutput")
    # DRAM scratch for the tail softmax-normalize broadcasts (must be an
    # ExternalOutput in this environment)
    dscr = nc.dram_tensor("dscr", [32, 512], F32, kind="ExternalOutput")

    with tile.TileContext(nc) as tc:
        with tc.tile_pool(name="persist", bufs=1) as persist:
            kropeT = persist.tile([R, S], BF16, tag="kropeT")
            t_ones = persist.tile([P, 1], BF16, tag="ones")
            nc.vector.memset(t_ones, 1.0)
            t_onesr = persist.tile([1, P], BF16, tag="onesr")
            nc.vector.memset(t_onesr, 1.0)
            qCT = persist.tile([P, HPG, S], BF16, tag="qCT")
            qrE = persist.tile([R, 2, S], BF16, tag="qrE")   # heads 0,2
            qrO = persist.tile([R, 2, S], BF16, tag="qrO")   # heads 1,3
            cKV = persist.tile([P, KL, S], BF16, tag="cKV")
            kCT = persist.tile([P, HPG, S], BF16, tag="kCT")
            vC = persist.tile([P, ST, GCOL], BF16, tag="vC")

            # phase-2/3/4 fixed weights: pool opened before phase 1 so their
            # DMAs can fire mid-phase-1 (once the x DMA queue has drained)
            stack_wfix2 = tc.tile_pool(name="wfix2", bufs=1)
            wfix2 = stack_wfix2.__enter__()
            t_wuk = [wfix2.tile([P, KL * P], BF16, tag=f"wuk{mo}",
                                name=f"wuk{mo}") for mo in range(HPG)]
            t_wuv = wfix2.tile([P, KL * GCOL], BF16, tag="wuv")
            t_wo = wfix2.tile([P, HPG * E], BF16, tag="wo")

            def fire_wukv_dmas():
                for mo in range(HPG):
                    nc.sync.dma_start(out=t_wuk[mo], in_=wUK[mo])
                nc.sync.dma_start(out=t_wuv, in_=wUV[:, :])

            def fire_wo_dma():
                nc.sync.dma_start(out=t_wo, in_=wO[:, :])

            with tc.tile_pool(name="tabp", bufs=1) as tabp:
                t_cos = tabp.tile([R, S], F32, tag="cos")
                t_sin = tabp.tile([R, S], F32, tag="sin")

                def rope_from_psum(pool, psum, base, scol, dst):
                    """dst[...] (64 x 512, bf16) = rope(psum[base:base+64]).

                    psum rows [base:base+32]=x1, [base+32:base+64]=x2 (host
                    permuted weight cols). dst = x*[c;c] + swap(x)*[-s;s].
                    TensorCopy may shift partitions; TensorTensor operands
                    must share a base partition, hence copy-then-mul."""
                    sl = slice(scol, scol + 512)
                    swp = pool.tile([R, 512], F32, tag="swp")
                    nc.vector.tensor_copy(out=swp[0:32, :],
                                          in_=psum[base + 32:base + R, :])
                    nc.vector.tensor_copy(out=swp[32:R, :],
                                          in_=psum[base:base + 32, :])
                    nc.vector.tensor_mul(out=swp, in0=swp, in1=t_sin[:, sl])
                    aln = pool.tile([R, 512], F32, tag="aln")
                    if base == 0:
                        nc.vector.tensor_mul(out=aln, in0=psum[0:R, :],
                                             in1=t_cos[:, sl])
                    else:
                        nc.vector.tensor_copy(out=aln,
                                              in_=psum[base:base + R, :])
                        nc.vector.tensor_mul(out=aln, in0=aln,
                                             in1=t_cos[:, sl])
                    nc.vector.tensor_add(out=dst, in0=aln, in1=swp)

                # ---- Phase 1: one pass over x^T -> c_KV^T, k_rope^T,
                #      q_C^T, q_rope^T (all SBUF-resident) ----
                with (
                    tc.tile_pool(name="xt", bufs=7) as xtp,
                    tc.tile_pool(name="wfix1", bufs=1) as wfix1,
                    tc.tile_pool(name="ropetmp", bufs=1) as ropetmp,
                    tc.tile_pool(name="ps_p1", bufs=6, space="PSUM") as ps_p1,
                    tc.tile_pool(name="ps_kr", bufs=1, space="PSUM") as ps_kr,
                ):
                    # DMA issue order tracks first use: wDKV[0], then the
                    # first x block, then the remaining weights.
                    t_wdkv = [wfix1.tile([P, KE * P], BF16, tag=f"wdkv{mo}",
                                         name=f"wdkv{mo}") for mo in range(KL)]
                    t_wkr = wfix1.tile([P, KE * R], BF16, tag="wkr")
                    t_wuq = [wfix1.tile([P, KE * P], BF16, tag=f"wuq{mo}",
                                        name=f"wuq{mo}") for mo in range(HPG)]
                    t_wqr = [wfix1.tile([P, KE * P], BF16, tag=f"wqr{mo}",
                                        name=f"wqr{mo}") for mo in range(2)]
                    nc.sync.dma_start(out=t_wqr[0], in_=wQR[0])

                    for sb in range(SBN):
                        ssl = slice(sb * 512, (sb + 1) * 512)
                        # 4 tiles per DMA: 4x fewer serialized HWDGE slots
                        xts = []
                        for g in range(4):
                            t = xtp.tile([P, 4, 512], BF16, tag="xt")
                            src0 = xT[g * 4 * P:g * 4 * P + P, ssl]
                            nc.sync.dma_start(out=t, in_=bass.AP(
                                tensor=src0.tensor, offset=src0.offset,
                                ap=[[S, P], [P * S, 4], [1, 512]]))
                            xts.extend(t[:, j, :] for j in range(4))
                        if sb == 0:
                            nc.sync.dma_start(out=t_wqr[1], in_=wQR[1])
                            nc.sync.dma_start(out=t_wkr, in_=wKR[:, :])
                            for mo in range(KL):
                                nc.sync.dma_start(out=t_wdkv[mo], in_=wDKV[mo])
                            for mo in range(HPG):
                                nc.sync.dma_start(out=t_wuq[mo], in_=wUQ[mo])
                            # rope tables gate only DVE work (slack there)
                            nc.sync.dma_start(out=t_cos, in_=cosq[:, :])
                            nc.sync.dma_start(out=t_sin, in_=sinq[:, :])
                            fire_wukv_dmas()
                        elif sb == 2:
                            fire_wo_dma()

                        for mo in range(2):    # q_rope^T (2 heads per chain)
                            psum = ps_p1.tile([P, 512], F32, tag="p")
                            for k in range(KE):
                                nc.tensor.matmul(
                                    psum, t_wqr[mo][:, k * P:(k + 1) * P], xts[k],
                                    start=(k == 0), stop=(k == KE - 1))
                            rope_from_psum(ropetmp, psum, 0, sb * 512,
                                           qrE[:, mo, ssl])
                            rope_from_psum(ropetmp, psum, R, sb * 512,
                                           qrO[:, mo, ssl])
                        # k_rope^T
                        psum = ps_kr.tile([R, 512], F32, tag="pkr")
                        for k in range(KE):
                            nc.tensor.matmul(
                                psum, t_wkr[:, k * R:(k + 1) * R], xts[k],
                                start=(k == 0), stop=(k == KE - 1))
                        rope_from_psum(ropetmp, psum, 0, sb * 512,
                                       kropeT[:, ssl])
                        for mo in range(KL):   # c_KV^T
                            psum = ps_p1.tile([P, 512], F32, tag="p")
                            for k in range(KE):
                                nc.tensor.matmul(
                                    psum, t_wdkv[mo][:, k * P:(k + 1) * P], xts[k],
                                    start=(k == 0), stop=(k == KE - 1))
                            nc.scalar.activation(out=cKV[:, mo, ssl],
                                                 in_=psum, func=Copy)
                        for mo in range(HPG):  # q_C^T
                            psum = ps_p1.tile([P, 512], F32, tag="p")
                            for k in range(KE):
                                nc.tensor.matmul(
                                    psum, t_wuq[mo][:, k * P:(k + 1) * P], xts[k],
                                    start=(k == 0), stop=(k == KE - 1))
                            if mo % 2 == 0:
                                nc.scalar.activation(out=qCT[:, mo, ssl],
                                                     in_=psum, func=Copy)
                            else:
                                nc.vector.tensor_copy(out=qCT[:, mo, ssl],
                                                      in_=psum)
                        for mo in range(HPG):  # k_C^T
                            psum = ps_p1.tile([P, 512], F32, tag="p")
                            for k in range(KL):
                                nc.tensor.matmul(
                                    psum, t_wuk[mo][:, k * P:(k + 1) * P],
                                    cKV[:, k, ssl],
                                    start=(k == 0), stop=(k == KL - 1))
                            if mo % 2 == 0:
                                nc.scalar.activation(out=kCT[:, mo, ssl],
                                                     in_=psum, func=Copy)
                            else:
                                nc.vector.tensor_copy(out=kCT[:, mo, ssl],
                                                      in_=psum)
                        for loc in range(4):   # v_C
                            st = sb * 4 + loc
                            psum = ps_p1.tile([P, GCOL], F32, tag="p")
                            for k in range(KL):
                                nc.tensor.matmul(
                                    psum, cKV[:, k, st * P:(st + 1) * P],
                                    t_wuv[:, k * GCOL:(k + 1) * GCOL],
                                    start=(k == 0), stop=(k == KL - 1))
                            if loc % 2 == 0:
                                nc.scalar.activation(out=vC[:, st, :],
                                                     in_=psum, func=Copy)
                            else:
                                nc.vector.tensor_copy(out=vC[:, st, :],
                                                      in_=psum)

            # ---- Phases 2+3 per seq block: k_C^T, v_C, then attention ----
            stack_p2 = tc.tile_pool(name="persist2", bufs=1)
            persist2 = stack_p2.__enter__()
            aoT = persist2.tile([P, HPG, S], BF16, tag="aoT")
            with (
                tc.tile_pool(name="wfix3", bufs=1) as wfix3,
                tc.tile_pool(name="att", bufs=4) as att,
                tc.tile_pool(name="attsm", bufs=3) as attsm,
                tc.tile_pool(name="ps_s", bufs=4, space="PSUM") as ps_s,
                tc.tile_pool(name="ps_o", bufs=2, space="PSUM") as ps_o,
                tc.tile_pool(name="ps_d", bufs=2, space="PSUM") as ps_d,
            ):
                t_mask = wfix3.tile([P, 4 * 512], F32, tag="mask")
                nc.sync.dma_start(out=t_mask, in_=maskin[:, :])

                # normalization of head block (h, ssl) is deferred until the
                # next head's score loop has been issued, hiding the
                # recip->copy->broadcast-matmul latency under PE work
                pend = []

                nbounce = [0]

                def normalize_pending(drow=None):
                    h, pssl, psum_o, psum_d = pend.pop()
                    if pssl.start >= 1024:
                        # long blocks: bounce off-PE; latency hides under
                        # the >8us of PE work before aoT is next needed
                        drow = 4 * nbounce[0]
                        nbounce[0] += 1
                    if drow is None:
                        rec32 = attsm.tile([1, 512], F32, tag="rec32")
                        nc.vector.reciprocal(out=rec32, in_=psum_d)
                        rec16 = attsm.tile([1, 512], BF16, tag="rec16")
                        nc.scalar.activation(out=rec16, in_=rec32, func=Copy)
                        psum_bc = ps_s.tile([P, 512], F32, tag="p")
                        nc.tensor.matmul(psum_bc, t_onesr, rec16,
                                         start=True, stop=True)
                        bc16 = attsm.tile([P, 512], BF16, tag="bc16")
                        nc.scalar.activation(out=bc16, in_=psum_bc, func=Copy)
                        nc.vector.tensor_tensor(aoT[:, h, pssl], psum_o,
                                                bc16, Mult)
                        return
                    # tail blocks: no PE work follows to hide the broadcast
                    # matmul, so bounce 1/denom through DRAM on the idle
                    # Pool-engine DMA path instead (keeps the PE stream dry)
                    tmp4 = attsm.tile([4, 512], F32, tag="tmp4")
                    nc.vector.memset(tmp4, 0.0)
                    nc.vector.tensor_copy(out=tmp4[0:1, :], in_=psum_d)
                    nc.vector.reciprocal(out=tmp4[0:1, :], in_=tmp4[0:1, :])
                    nc.sync.dma_start(out=dscr[drow:drow + 4, :], in_=tmp4)
                    bc = attsm.tile([P, 512], F32, tag="bc")
                    srcr = dscr[drow, :]
                    nc.gpsimd.dma_start(out=bc, in_=bass.AP(
                        tensor=srcr.tensor, offset=srcr.offset,
                        ap=[[0, P]] + [list(x) for x in srcr.ap]))
                    nc.vector.tensor_tensor(aoT[:, h, pssl], psum_o, bc, Mult)

                for sb in range(SBN):
                    ssl = slice(sb * 512, (sb + 1) * 512)
                    # attention for query block sb over key tiles 0..4sb+3
                    T = 4 * (sb + 1)
                    for h in range(HPG):
                        psum_o = ps_o.tile([P, 512], F32, tag="p")
                        psum_d = ps_d.tile([1, 512], F32, tag="p")
                        for tt in range(T):
                            tsl = slice(tt * P, (tt + 1) * P)
                            r = tt - 4 * sb
                            # diagonal key tiles: queries < 128r of this
                            # block are fully masked -> trim the free dim
                            qo = P * r if r > 0 else 0
                            qs = slice(qo, 512)
                            gqs = slice(sb * 512 + qo, (sb + 1) * 512)
                            hqs = (qrE if h % 2 == 0 else qrO)[:, h // 2, gqs]
                            sg = qo > 0
                            psum_s = ps_s.tile([P, 512], F32, tag="p")
                            nc.tensor.matmul(psum_s[:, qs], kCT[:, h, tsl],
                                             qCT[:, h, gqs],
                                             start=True, stop=False)
                            nc.tensor.matmul(psum_s[:, qs], kropeT[:, tsl],
                                             hqs, start=False, stop=True)
                            expT = att.tile([P, 512], BF16, tag="expT")
                            if r >= 0:
                                masked = attsm.tile([P, 512], BF16,
                                                    tag="masked")
                                nc.vector.tensor_add(
                                    out=masked[:, qs], in0=psum_s[:, qs],
                                    in1=t_mask[:, r * 512 + qo:(r + 1) * 512])
                                nc.scalar.activation(out=expT[:, qs],
                                                     in_=masked[:, qs],
                                                     func=Exp)
                            else:
                                nc.scalar.activation(out=expT, in_=psum_s,
                                                     func=Exp)
                            nc.tensor.matmul(psum_d[:, qs], t_ones,
                                             expT[:, qs],
                                             start=(tt == 0), stop=(tt == T - 1),
                                             skip_group_check=sg)
                            nc.tensor.matmul(psum_o[:, qs],
                                             vC[:, tt, h * DH:(h + 1) * DH],
                                             expT[:, qs],
                                             start=(tt == 0), stop=(tt == T - 1),
                                             skip_group_check=sg)
                        if pend:
                            normalize_pending()
                        pend.append((h, ssl, psum_o, psum_d))
                if pend:
                    normalize_pending()

            # ---- Phase 4: out^T partial = W_O[g]^T @ aoT ----
            with (
                tc.tile_pool(name="oout", bufs=6) as oout,
                tc.tile_pool(name="ps_w", bufs=4, space="PSUM") as ps_w,
            ):
                for sb in range(SBN):
                    ssl = slice(sb * 512, (sb + 1) * 512)
                    for mo in range(KE):
                        psum_w = ps_w.tile([P, 512], F32, tag="p")
                        for k in range(HPG):
                            nc.tensor.matmul(psum_w,
                                             t_wo[:, k * E + mo * P:k * E + (mo + 1) * P],
                                             aoT[:, k, ssl],
                                             start=(k == 0), stop=(k == HPG - 1))
                        ot = oout.tile([P, 512], BF16, tag="oout")
                        if mo % 2 == 0:
                            nc.scalar.activation(out=ot, in_=psum_w, func=Copy)
                        else:
                            nc.vector.tensor_copy(out=ot, in_=psum_w)
                        nc.sync.dma_start(
                            out=outT[mo * P:(mo + 1) * P, ssl], in_=ot)
            stack_p2.__exit__(None, None, None)
            stack_wfix2.__exit__(None, None, None)

    nc.compile()
    return nc


def _host_inputs(inputs):
    """Per-core input maps (host-side sharding + weight folding/pre-tiling)."""
    f32 = np.float32
    bf16 = mybir.dt.np(BF16)
    x = inputs["x"]
    W_DQ = inputs["W_DQ"].astype(f32)
    W_UQ, W_QR = inputs["W_UQ"].astype(f32), inputs["W_QR"].astype(f32)
    W_DKV, W_UK = inputs["W_DKV"].astype(f32), inputs["W_UK"].astype(f32)
    W_KR, W_UV = inputs["W_KR"].astype(f32), inputs["W_UV"].astype(f32)
    W_O = inputs["W_O"].astype(f32)

    # fold the shared down-projection into the query weights
    WUQf = (W_DQ @ W_UQ) * SCALE          # [E, E]
    WQRf = (W_DQ @ W_QR) * SCALE          # [E, R*H]

    # shared across cores
    wDKV_t = _lhsT_layout(W_DKV).astype(bf16).reshape(KL, P, KE * P)
    wKR_t = _rhs_layout(_rope_perm_cols(W_KR)).astype(bf16).reshape(P, KE * R)
    half = R // 2
    freqs = BASE ** (-np.arange(half, dtype=np.float64) / half)
    theta = np.arange(S, dtype=np.float64)[None, :] * freqs[:, None]   # [32, S]
    cos2 = np.concatenate([np.cos(theta), np.cos(theta)], 0).astype(f32)
    sinpm = np.concatenate([-np.sin(theta), np.sin(theta)], 0).astype(f32)
    p = np.arange(P)[:, None, None]
    rr = np.arange(4)[None, :, None]
    f = np.arange(512)[None, None, :]
    maskadd = np.where(p <= f - P * rr, 0.0, NEG).astype(f32).reshape(P, 4 * 512)

    in_maps = []
    for c in range(8):
        b, g = divmod(c, 4)
        cs, ce = g * GCOL, (g + 1) * GCOL          # head cols of this group
        wUQ_g = _lhsT_layout(WUQf[:, cs:ce]).astype(bf16).reshape(HPG, P, KE * P)
        qr = WQRf[:, g * HPG * R:(g + 1) * HPG * R]
        wQR_g = _lhsT_layout(_rope_perm_cols(qr)).astype(bf16).reshape(2, P, KE * P)
        wUK_g = _lhsT_layout(W_UK[:, cs:ce]).astype(bf16).reshape(HPG, P, KL * P)
        wUV_g = _rhs_layout(W_UV[:, cs:ce]).astype(bf16).reshape(P, KL * GCOL)
        wO_g = _rhs_layout(W_O[cs:ce, :]).astype(bf16).reshape(P, HPG * E)
        in_maps.append({
            "xT": np.ascontiguousarray(x[b].T).astype(bf16),
            "wUQ": wUQ_g, "wQR": wQR_g, "wDKV": wDKV_t,
            "wUK": wUK_g, "wUV": wUV_g, "wKR": wKR_t, "wO": wO_g,
            "cosq": cos2, "sinq": sinpm, "maskin": maskadd,
        })
    return in_maps


def _assemble(results):
    out = np.empty((B, S, E), np.float32)
    for b in range(B):
        acc = results[4 * b]["outT"].astype(np.float32).copy()
        for g in range(1, 4):
            acc += results[4 * b + g]["outT"]
        out[b] = acc.T
    return out


def kernel(**inputs):
    inputs = {k: np.asarray(v) for k, v in inputs.items()}
    if "nc" not in _CACHE:
        _CACHE["nc"] = build_nc()
    nc = _CACHE["nc"]
    in_maps = _host_inputs(inputs)
    res = run_bass_kernel_spmd(nc, in_maps, core_ids=list(range(8)))
    return _assemble(res.results)
